# revision 39
# baseline (speedup 1.0000x reference)
"""Trainium2 Bass kernel for nn_MultiHeadAttention (dense transformer block:
qkv proj + RoPE + causal SDPA + out proj), tensor-parallel over (batch, heads)
across 8 NeuronCores.

Sharding: 2 batches x 16 heads = 32 (b,h) pairs; core c handles batch c//4,
heads 4*(c%4)..4*(c%4)+3. Each core computes qkv for its 4 heads (from the
full x of its batch), RoPE, causal attention, and a PARTIAL output
projection (its heads' rows of Wproj); the host sums the 4 partials per
batch.

v2: all four big GEMMs (q/k/v projections and the output projection) run in
fp8e4m3 DoubleRow mode (2 contraction tiles per instruction, 0.5 cyc/row)
with a 3-term hi/lo error-compensated split:
    x @ W ~= (XA@WA + XA@WB + XC@WC) / 1024
where XA=fp8(x), XC=fp8(16*(x-XA)), WA=fp8(16*c*W), WC=fp8(c*W),
WB=fp8(16*(c*W - WC)), c=64. This keeps quantization error at bf16 level
(~0.1% per GEMM) while running the PE 1.33x faster than bf16 on those GEMMs.
Attention (scores / exp / PV) stays bf16.

Layout notes:
- x is passed pre-transposed per batch and pre-split into fp8 hi/lo (XA/XC
  [D, S]) so the contraction dim lands on SBUF partitions, streamed through
  SBUF in 512-token chunks.
- q/k head dims are permuted host-side into a 16-interleaved (even,odd)
  order so RoPE's pair swap is a quadrant-local DVE stream_shuffle.
  Attention scores are invariant to this (q and k permuted identically).
- RoPE tables carry sqrt(scale)/1024 (the fp8 descale); V stays at 1024x in
  bf16 and the 1/1024 is folded into the host-side Wproj scaling.
- Scores are computed transposed (S^T [kv, q]) so softmax's denominator
  comes from a ones-matmul (column sums) and P^T feeds the O = V^T @ P^T
  matmul directly. exp() runs without max-subtraction: |scores| < ~10 for
  this input distribution, safe in fp32.
- The attention output is renormalized into T1 = 32*y_head (ones matrix
  holds 32.0 so rinv = 1/(32*l)), then split into fp8 hi/lo (YA/YC) feeding
  the fp8 out-projection; final psum carries 32768*out, descaled by the
  ACT copy.
"""
import sys

sys.path.insert(0, "/opt/trn_rl_repo")

from collections import deque

import numpy as np
import ml_dtypes

import concourse.bass as bass
import concourse.mybir as mybir
import concourse.tile as tile

P = 128
B, S, D = 2, 2048, 2048
NH, HD = 16, 128
NH_CORE = 4  # heads per core
HCOLS = NH_CORE * HD  # 512
KT = D // P  # 16 k-tiles
KP = KT // 2  # 8 k-tile pairs (DoubleRow)
TT = S // P  # 16 token tiles
QC = 512  # q-chunk width
NQC = S // QC  # 4
ROPE_THETA = 10000.0
SCALE = HD**-0.5
NEG = -30000.0

F32 = mybir.dt.float32
BF16 = mybir.dt.bfloat16
F8 = mybir.dt.float8e4
DR = mybir.MatmulPerfMode.DoubleRow

F8NP = ml_dtypes.float8_e4m3

# fp8 scale bookkeeping:
#   qkv:   XA(1) @ WA(1024) etc -> psum = 1024 * qkv
#   rope tables carry sqrt(SCALE)/1024  -> Qt/Kt = sqrt(SCALE) * q/k
#   Vt (bf16) = 1024 * v
#   ones matrix = 32.0 -> rinv = 1/(32*l) -> T1 = o_ps*rinv = 32*y_head
#   outproj: YA(32) @ WpA(1024/32*16... ) -> psum = 32768*out
OUT_DESCALE = 1.0 / 32768.0

_SWAP16 = [(i + 16) % 32 for i in range(32)]


# ---------------------------------------------------------------------------
# host-side constant tables
# ---------------------------------------------------------------------------
def _dim_perm():
    """Permutation p -> original head-dim index, 16-interleaved even/odd."""
    perm = np.zeros(HD, dtype=np.int64)
    for p in range(HD):
        qd, sl = p // 32, p % 32
        i = 16 * qd + (sl % 16)
        perm[p] = 2 * i if sl < 16 else 2 * i + 1
    return perm


def _rope_tables():
    """ctab[p,t], stab[p,t] (sign-baked) for the permuted head-dim layout."""
    perm = _dim_perm()
    inv_freq = 1.0 / (ROPE_THETA ** (np.arange(0, HD, 2, dtype=np.float64) / HD))
    t = np.arange(S, dtype=np.float64)
    ctab = np.zeros((HD, S), dtype=np.float64)
    stab = np.zeros((HD, S), dtype=np.float64)
    for p in range(HD):
        qd, sl = p // 32, p % 32
        i = 16 * qd + (sl % 16)
        ang = t * inv_freq[i]
        ctab[p] = np.cos(ang)
        stab[p] = -np.sin(ang) if sl < 16 else np.sin(ang)
    return ctab.astype(np.float32), stab.astype(np.float32)


def _tri_mask():
    """[P, P] f32: 0 where kv(row) <= q(col) else NEG."""
    b = np.arange(P)[:, None]
    a = np.arange(P)[None, :]
    return np.where(b <= a, 0.0, NEG).astype(np.float32)


def _split3_w(W, c):
    """3-term fp8 split of weights: WA=fp8(16c*W), WB=fp8(16*(c*W-fp8(c*W))),
    WC=fp8(c*W). All numpy fp8e4m3."""
    Ws = (c * W).astype(np.float32)
    WC = Ws.astype(F8NP)
    WB = (16.0 * (Ws - WC.astype(np.float32))).astype(F8NP)
    WA = (16.0 * Ws).astype(F8NP)
    return WA, WB, WC


def _split2_x(x):
    """XA=fp8(x), XC=fp8(16*(x-XA))."""
    XA = x.astype(F8NP)
    XC = (16.0 * (x - XA.astype(np.float32))).astype(F8NP)
    return XA, XC


# ---------------------------------------------------------------------------
# device kernel
# ---------------------------------------------------------------------------
def _build_nc():
    nc = bass.Bass()

    XA = nc.declare_dram_parameter("XA", [D, S], F8, isOutput=False)
    XC = nc.declare_dram_parameter("XC", [D, S], F8, isOutput=False)
    wq = [nc.declare_dram_parameter(f"Wq{t}", [D, HCOLS], F8, isOutput=False)
          for t in "ABC"]
    wk = [nc.declare_dram_parameter(f"Wk{t}", [D, HCOLS], F8, isOutput=False)
          for t in "ABC"]
    wv = [nc.declare_dram_parameter(f"Wv{t}", [D, HCOLS], F8, isOutput=False)
          for t in "ABC"]
    wp = [nc.declare_dram_parameter(f"Wp{t}", [HCOLS, D], F8, isOutput=False)
          for t in "ABC"]
    out = nc.declare_dram_parameter("out", [S, D], BF16, isOutput=True)

    # sqrt(SCALE)/1024 on both q and k tables => scores scaled by SCALE
    ctab_np, stab_np = _rope_tables()
    rt = np.float32(np.sqrt(SCALE) / 1024.0)
    cq_d = nc.inline_tensor((ctab_np * rt).astype(ml_dtypes.bfloat16), "cq")
    sq_d = nc.inline_tensor((stab_np * rt).astype(ml_dtypes.bfloat16), "sq")
    # transposed causal mask (bf16) for the PE psum-preload trick, plus a
    # bf16 identity as its moving operand
    trimT_d = nc.inline_tensor(
        np.ascontiguousarray(_tri_mask().T).astype(ml_dtypes.bfloat16),
        "trimT",
    )
    identb_d = nc.inline_tensor(
        np.eye(P, dtype=np.float32).astype(ml_dtypes.bfloat16), "identb"
    )
    ident_d = nc.inline_tensor(np.eye(P, dtype=np.float32), "ident")
    # indicator for the rinv broadcast matmuls; carries the 1/32 T1 scale
    ind_np = np.zeros((4, QC), dtype=np.float32)
    for s in range(4):
        ind_np[s, s * P:(s + 1) * P] = 1.0 / 32.0
    ind_d = nc.inline_tensor(ind_np.astype(ml_dtypes.bfloat16), "indic")

    XA_t = XA[:].rearrange("(ko p) t -> p ko t", p=P)
    XC_t = XC[:].rearrange("(ko p) t -> p ko t", p=P)
    wq_t = [w[:].rearrange("(ko p) m -> p ko m", p=P) for w in wq]
    wk_t = [w[:].rearrange("(ko p) m -> p ko m", p=P) for w in wk]
    wv_t = [w[:].rearrange("(ko p) m -> p ko m", p=P) for w in wv]
    wp_t = [w[:].rearrange("(ho p) n -> p ho n", p=P) for w in wp]
    out_t = out[:].rearrange("(to p) n -> p to n", p=P)

    with tile.TileContext(nc) as tc:
        with (
            tc.tile_pool(name="persist", bufs=1) as pp,
            tc.tile_pool(name="work", bufs=2) as wkp,
            tc.tile_pool(name="pt", bufs=3) as ptp,
            tc.tile_pool(name="t1", bufs=1) as t1p,
            tc.tile_pool(name="qtp", bufs=2) as qtp,
            tc.tile_pool(name="xwin", bufs=2) as xw,
            tc.tile_pool(name="outp", bufs=3) as outp,
            tc.tile_pool(name="ps_g", bufs=2, space="PSUM") as psG,
            tc.tile_pool(name="ps_s", bufs=3, space="PSUM") as psS,
            tc.tile_pool(name="ps_o", bufs=2, space="PSUM") as psO,
            tc.tile_pool(name="ps_l", bufs=1, space="PSUM") as psL,
        ):
            # PE warmup: keep the tensor engine busy during the initial DMAs
            # so the p-state ramp finishes before real work starts.
            warm = pp.tile([P, P], BF16)
            nc.vector.memset(warm, 0.0)
            wps = psG.tile([P, QC], F32, tag="ps")
            for i in range(48):
                nc.tensor.matmul(wps[:, 0:P], warm, warm, start=(i == 0),
                                 stop=(i == 47))

            # persistent tiles
            cq = pp.tile([P, S], BF16)
            sq = pp.tile([P, S], BF16)
            trimT = pp.tile([P, P], BF16)
            identb = pp.tile([P, P], BF16)
            ident_sb = pp.tile([P, P], F32)
            ind_sb = pp.tile([4, QC], BF16)
            ones_col = pp.tile([P, 1], BF16)
            nc.vector.memset(ones_col, 1.0)

            Kt = pp.tile([P, NH_CORE, S], BF16)
            Vt = pp.tile([P, TT, HCOLS], BF16)
            YA = pp.tile([P, NH_CORE, S], F8)
            YC = pp.tile([P, NH_CORE, S], F8)

            wq_sb = [pp.tile([P, KT, HCOLS], F8, name=f"wq{t}")
                     for t in "ABC"]
            wk_sb = [pp.tile([P, KT, HCOLS], F8, name=f"wk{t}")
                     for t in "ABC"]
            wv_sb = [pp.tile([P, KT, HCOLS], F8, name=f"wv{t}")
                     for t in "ABC"]
            wp_sb = [pp.tile([P, NH_CORE, D], F8, name=f"wp{t}")
                     for t in "ABC"]
            # spread the initial loads across SP/Pool/ACT so the first
            # matmuls (A-term of the k-projection) can start ~3.5us in.
            # Pool stays mostly free (it drains proj psums from ~7us on);
            # ACT is free until the first exp (~23us).
            nc.gpsimd.dma_start(wk_sb[0], wk_t[0])
            nc.gpsimd.dma_start(wk_sb[2], wk_t[2])
            nc.scalar.dma_start(wk_sb[1], wk_t[1])
            nc.scalar.dma_start(cq, cq_d[:])
            nc.scalar.dma_start(sq, sq_d[:])
            nc.scalar.dma_start(wv_sb[0], wv_t[0])
            nc.scalar.dma_start(wv_sb[1], wv_t[1])
            nc.scalar.dma_start(wq_sb[1], wq_t[1])

            def load_xchunk(tcx):
                xa = xw.tile([P, KT, QC], F8, tag="xa")
                xc = xw.tile([P, KT, QC], F8, tag="xc")
                nc.sync.dma_start(xa, XA_t[:, :, tcx * QC:(tcx + 1) * QC])
                nc.sync.dma_start(xc, XC_t[:, :, tcx * QC:(tcx + 1) * QC])
                return xa, xc

            def mm3(ps, w3, x2, lhs_w, hsl, xsl):
                """24 DoubleRow matmuls accumulating the 3-term product.
                lhs_w: True if weights are the stationary operand."""
                terms = ((w3[0], x2[0]), (w3[1], x2[0]), (w3[2], x2[1]))
                n = len(terms) * KP
                i = 0
                for wt, xt in terms:
                    for kp in range(KP):
                        ks = slice(2 * kp, 2 * kp + 2)
                        if lhs_w:
                            lhsT = wt[:, ks, hsl]
                            rhs = xt[:, ks, xsl]
                        else:
                            lhsT = xt[:, ks, xsl]
                            rhs = wt[:, ks, hsl]
                        nc.tensor.matmul(
                            ps, lhsT, rhs,
                            start=(i == 0), stop=(i == n - 1),
                            perf_mode=DR,
                        )
                        i += 1

            def rope(ps, dst, csl):
                pc = wkp.tile([P, QC], BF16, tag="pc")
                nc.gpsimd.tensor_copy(pc, ps)
                xsw = wkp.tile([P, QC], BF16, tag="xsw")
                nc.vector.stream_shuffle(xsw, pc, _SWAP16)
                nc.vector.tensor_mul(pc, pc, cq[:, csl])
                nc.vector.tensor_mul(xsw, xsw, sq[:, csl])
                nc.vector.tensor_add(dst, pc, xsw)

            # softmax-denominator bank: l columns [0:4] reused every chunk
            # (the WAR overlap with the previous reciprocal read orders the
            # bank-zeroing start correctly).
            lps = psL.tile([P, QC], F32)
            chunks = [(h, qc) for qc in range(NQC) for h in range(NH_CORE)]
            o_hist, rv_hist, rvT_hist, bc_hist = {}, {}, {}, {}
            pts = {}

            def post_a(i):
                # rinv [128,4] -> transposed [4,128] block in a psG slot
                rt_ps = psG.tile([P, P], F32, tag="ps")
                nc.tensor.transpose(rt_ps[0:4], rv_hist[i], ident_sb)
                rvT = wkp.tile([P, P], BF16, tag="rvT")
                nc.scalar.activation(
                    rvT[0:4], rt_ps[0:4],
                    mybir.ActivationFunctionType.Copy,
                )
                rvT_hist[i] = rvT

            def post_b(i):
                # broadcast rinv/32 across partitions: indicator matmuls
                bc_ps = psS.tile([P, QC], F32, tag="sps")
                for s in range(4):
                    nc.tensor.matmul(
                        bc_ps[:, s * P:(s + 1) * P],
                        ind_sb[0:4, s * P:(s + 1) * P],
                        rvT_hist[i][0:4],
                        start=(s == 0),
                        stop=(s == 3),
                        skip_group_check=True,
                    )
                bc = wkp.tile([P, QC], BF16, tag="bc", bufs=1)
                nc.gpsimd.tensor_copy(bc, bc_ps)
                bc_hist[i] = bc

            def post_c(i):
                h, qc = chunks[i]
                qsl = slice(qc * QC, (qc + 1) * QC)
                # T1 = 32 * y_head (f32), then fp8 hi/lo split
                t1 = t1p.tile([P, QC], F32, tag="t1")
                nc.vector.tensor_mul(t1, o_hist[i], bc_hist[i])
                nc.gpsimd.tensor_copy(YA[:, h, qsl], t1)
                nc.vector.tensor_sub(t1, t1, YA[:, h, qsl])
                nc.gpsimd.tensor_scalar_mul(YC[:, h, qsl], t1, 16.0)

            # Cross-chunk software pipeline: scores/exp run ~3 jb ahead of
            # PV/l so the mask+exp latency hides under later scores matmuls.
            # `pending` holds (chunk, jb) pairs whose PV/l is not yet
            # emitted; `after_pop` holds the deferred rinv post-chain steps,
            # one drained per pop so they spread between PV matmuls.
            pending = deque()
            after_pop = deque()

            def pv_l(i, jb):
                h, qc = chunks[i]
                njb = 4 * qc + 4
                d = jb - 4 * qc
                off = 128 * d if d > 0 else 0
                pt = pts.pop((i, jb))
                nc.tensor.matmul(
                    o_hist[i][:, off:],
                    Vt[:, jb, h * HD:(h + 1) * HD],
                    pt[:, off:],
                    start=(jb == 0),
                    stop=(jb == njb - 1),
                )
                for s in range(max(d, 0), 4):
                    nc.tensor.matmul(
                        lps[:, s:s + 1],
                        pt[:, s * P:(s + 1) * P],
                        ones_col,
                        start=(jb == 0 and s == 0),
                        stop=(jb == 4 * qc + s),
                        skip_group_check=True,
                    )

            def pop_one():
                i, jb = pending.popleft()
                pv_l(i, jb)
                h, qc = chunks[i]
                if jb == 4 * qc + 3:  # chunk complete: reciprocal + posts
                    rv = wkp.tile([P, 4], F32, tag="rv")
                    nc.vector.reciprocal(rv, lps[:, 0:4])
                    rv_hist[i] = rv
                    after_pop.append(lambda i=i: post_a(i))
                    after_pop.append(lambda i=i: post_b(i))
                    after_pop.append(lambda i=i: post_c(i))
                elif after_pop:
                    after_pop.popleft()()

            def flush():
                while pending:
                    pop_one()
                while after_pop:
                    after_pop.popleft()()

            def attention_chunk(i, qtile):
                h, qc = chunks[i]
                o_hist[i] = psO.tile([P, QC], F32, tag="ops", name="o_ps")
                njb = 4 * qc + 4
                for jb in range(njb):
                    d = jb - 4 * qc  # diag offset if >= 0
                    off = 128 * d if d > 0 else 0
                    s_ps = psS.tile([P, QC], F32, tag="sps")
                    if d >= 0:
                        # causal mask preloaded into psum by the PE itself
                        # (start zeroes the bank, then writes the -30000s)
                        nc.tensor.matmul(
                            s_ps[:, off:off + P], trimT, identb,
                            start=True, stop=False, skip_group_check=True,
                        )
                        nc.tensor.matmul(
                            s_ps[:, off:],
                            Kt[:, h, jb * P:(jb + 1) * P],
                            qtile[:, h, off:],
                            start=False, stop=True, skip_group_check=True,
                        )
                    else:
                        nc.tensor.matmul(
                            s_ps[:, off:],
                            Kt[:, h, jb * P:(jb + 1) * P],
                            qtile[:, h, off:],
                            start=True, stop=True,
                        )
                    pt = ptp.tile([P, QC], BF16, tag="pt")
                    nc.scalar.activation(
                        pt[:, off:],
                        s_ps[:, off:],
                        mybir.ActivationFunctionType.Exp,
                    )
                    pts[(i, jb)] = pt
                    pending.append((i, jb))
                    if len(pending) > 2:
                        pop_one()

            def outproj_tt(tt):
                tsl = slice(tt * P, (tt + 1) * P)
                for ncx in range(D // QC):
                    nsl = slice(ncx * QC, (ncx + 1) * QC)
                    ps = psS.tile([P, QC], F32, tag="sps")
                    terms = ((YA, wp_sb[0]), (YA, wp_sb[1]), (YC, wp_sb[2]))
                    i = 0
                    for yt, wt in terms:
                        for hp in range(NH_CORE // 2):
                            hs = slice(2 * hp, 2 * hp + 2)
                            nc.tensor.matmul(
                                ps,
                                yt[:, hs, tsl],
                                wt[:, hs, nsl],
                                start=(i == 0),
                                stop=(i == 5),
                                perf_mode=DR,
                            )
                            i += 1
                    ob = outp.tile([P, QC], BF16, tag="ob")
                    nc.gpsimd.tensor_scalar_mul(ob, ps, float(OUT_DESCALE))
                    eng = nc.sync if (tt * 4 + ncx) % 2 == 0 else nc.scalar
                    eng.dma_start(out_t[:, tt, nsl], ob)

            # ---- merged projection + attention + out-proj, tcx-major ------
            # chunk (h, qc) only needs K/V tiles up to tcx=qc and Q(h, qc),
            # so projections for tcx and attention for qc=tcx interleave;
            # out-projection for qc's tokens runs during tcx=qc+1. The
            # ACT-heavy attention overlaps the PE-heavy projections.
            ci = 0
            xa_c, xc_c = load_xchunk(0)
            # remaining prologue loads, behind x chunk 0 on SP
            nc.sync.dma_start(ident_sb, ident_d[:])
            nc.sync.dma_start(trimT, trimT_d[:])
            nc.sync.dma_start(identb, identb_d[:])
            nc.sync.dma_start(ind_sb[0:4], ind_d[:])
            nc.sync.dma_start(wv_sb[2], wv_t[2])
            nc.sync.dma_start(wq_sb[2], wq_t[2])
            nc.sync.dma_start(wq_sb[0], wq_t[0])
            for i in range(3):
                nc.sync.dma_start(wp_sb[i], wp_t[i])
            for tcx in range(NQC):
                if tcx + 1 < NQC:
                    xa_n, xc_n = load_xchunk(tcx + 1)
                csl = slice(tcx * QC, (tcx + 1) * QC)
                for h in range(NH_CORE):  # k projection + RoPE
                    ps = psG.tile([P, QC], F32, tag="ps")
                    mm3(ps, wk_sb, (xa_c, xc_c), True,
                        slice(h * HD, (h + 1) * HD), slice(None))
                    rope(ps, Kt[:, h, csl], csl)
                for sub in range(4):  # v projection (1024x scale)
                    tt = 4 * tcx + sub
                    ps = psG.tile([P, HCOLS], F32, tag="ps")
                    mm3(ps, wv_sb, (xa_c, xc_c), False,
                        slice(None), slice(sub * P, (sub + 1) * P))
                    nc.gpsimd.tensor_copy(Vt[:, tt], ps)
                qtile = qtp.tile([P, NH_CORE, QC], BF16, tag="qt")
                for h in range(NH_CORE):  # q projection + RoPE
                    ps = psG.tile([P, QC], F32, tag="ps")
                    mm3(ps, wq_sb, (xa_c, xc_c), True,
                        slice(h * HD, (h + 1) * HD), slice(None))
                    rope(ps, qtile[:, h], csl)
                for h in range(NH_CORE):  # attention chunks for qc=tcx,
                    attention_chunk(ci, qtile)  # out-proj for qc=tcx-1
                    ci += 1
                    if tcx > 0:
                        outproj_tt(4 * (tcx - 1) + h)
                flush()
                if tcx + 1 < NQC:
                    xa_c, xc_c = xa_n, xc_n
            for tt in range(4 * (NQC - 1), TT):  # last qc's out-projection
                outproj_tt(tt)
    return nc


# ---------------------------------------------------------------------------
# legalization: this walrus build supports only ONE sync wait per instruction
# ---------------------------------------------------------------------------
_ENGINE_SEM_PREFIX = {
    "PE": "PE_",
    "DVE": "DVE_",
    "ACT": "ACT_",
    "Pool": "POOL_",
    "SP": "SP_",
}
_wf_counter = [0]


def _legalize(nc, max_waits=1):
    for f in nc.m.functions:
        for bb in f.blocks:
            new_insts = []
            for inst in bb.instructions:
                si = getattr(inst, "sync_info", None)
                eng = getattr(inst, "engine", None)
                if si is None or not si.on_wait or eng is None:
                    new_insts.append(inst)
                    continue
                waits = list(si.on_wait)
                pref = _ENGINE_SEM_PREFIX.get(eng.name)
                if pref is not None:
                    waits = [
                        w
                        for w in waits
                        if not (
                            w.sync_type == "semaphore"
                            and w.ant_name.startswith(pref)
                        )
                    ]
                if len(waits) > max_waits:
                    for w in waits[:-max_waits]:
                        _wf_counter[0] += 1
                        nop = mybir.InstNoOp(
                            name=f"I-waitfix-{_wf_counter[0]}", ins=[], outs=[]
                        )
                        nop.engine = eng
                        nop.sync_info = mybir.SyncInfo(on_wait=[w], on_update=[])
                        new_insts.append(nop)
                    waits = waits[-max_waits:]
                if len(waits) != len(si.on_wait):
                    inst.sync_info = mybir.SyncInfo(
                        on_wait=waits, on_update=list(si.on_update)
                    )
                new_insts.append(inst)
            bb.instructions[:] = new_insts


# ---------------------------------------------------------------------------
# SPMD runner (mirrors concourse.bass2jax.run_bass_via_pjrt, kept resident)
# ---------------------------------------------------------------------------
class _Runner:
    def __init__(self, nc, n_cores=8):
        import jax
        from jax.sharding import Mesh, PartitionSpec
        from jax.experimental.shard_map import shard_map
        from concourse import bass2jax
        from concourse.bass2jax import _bass_exec_p, install_neuronx_cc_hook

        install_neuronx_cc_hook()
        self.jax = jax
        self.nc = nc
        self.n_cores = n_cores
        partition_name = (
            nc.partition_id_tensor.name if nc.partition_id_tensor else None
        )
        in_names, out_names, out_avals, zero_outs = [], [], [], []
        for alloc in nc.m.functions[0].allocations:
            if not isinstance(alloc, mybir.MemoryLocationSet):
                continue
            name = alloc.memorylocations[0].name
            if alloc.kind == "ExternalInput":
                if name != partition_name:
                    in_names.append(name)
            elif alloc.kind == "ExternalOutput":
                shape = tuple(alloc.tensor_shape)
                dtype = mybir.dt.np(alloc.dtype)
                out_names.append(name)
                out_avals.append(jax.core.ShapedArray(shape, dtype))
                zero_outs.append(np.zeros(shape, dtype))
        self.in_names, self.out_names = in_names, out_names
        self.out_avals, self.zero_outs = out_avals, zero_outs
        n_params, n_outs = len(in_names), len(out_names)
        all_in_names = in_names + out_names
        if partition_name is not None:
            all_in_names.append(partition_name)
        donate = tuple(range(n_params, n_params + n_outs))

        def _body(*args):
            operands = list(args)
            if partition_name is not None:
                operands.append(bass2jax.partition_id_tensor())
            return tuple(
                _bass_exec_p.bind(
                    *operands,
                    out_avals=tuple(out_avals),
                    in_names=tuple(all_in_names),
                    out_names=tuple(out_names),
                    lowering_input_output_aliases=(),
                    sim_require_finite=True,
                    sim_require_nnan=True,
                    nc=nc,
                )
            )

        devices = jax.devices()[:n_cores]
        mesh = Mesh(np.asarray(devices), ("core",))
        in_specs = (PartitionSpec("core"),) * (n_params + n_outs)
        out_specs = (PartitionSpec("core"),) * n_outs
        self.fn = jax.jit(
            shard_map(
                _body,
                mesh=mesh,
                in_specs=in_specs,
                out_specs=out_specs,
                check_rep=False,
            ),
            donate_argnums=donate,
            keep_unused=True,
        )

    def run(self, in_maps):
        n = self.n_cores
        concat_in = [
            np.concatenate(
                [np.asarray(in_maps[c][name]) for c in range(n)], axis=0
            )
            for name in self.in_names
        ]
        zeros = [
            np.zeros((n * z.shape[0], *z.shape[1:]), z.dtype)
            for z in self.zero_outs
        ]
        out_arrs = self.fn(*concat_in, *zeros)
        return [
            {
                name: np.asarray(out_arrs[i]).reshape(
                    n, *self.out_avals[i].shape
                )[c]
                for i, name in enumerate(self.out_names)
            }
            for c in range(n)
        ]


_RUNNER = None


def _get_runner():
    global _RUNNER
    if _RUNNER is None:
        nc = _build_nc()
        _legalize(nc)
        _RUNNER = _Runner(nc, 8)
    return _RUNNER


# ---------------------------------------------------------------------------
# public entry point
# ---------------------------------------------------------------------------
def kernel(x, Wqkv, Wproj):
    x = np.asarray(x, dtype=np.float32)
    Wqkv = np.asarray(Wqkv, dtype=np.float32)
    Wproj = np.asarray(Wproj, dtype=np.float32)
    perm = _dim_perm()

    xsplit = [_split2_x(np.ascontiguousarray(x[b].T)) for b in range(B)]
    in_maps = []
    for c in range(8):
        b, g = c // 4, c % 4
        heads = range(NH_CORE * g, NH_CORE * (g + 1))
        qcols = np.concatenate([h * HD + perm for h in heads])
        WqA, WqB, WqC = _split3_w(Wqkv[:, 0 * D + qcols], 64.0)
        WkA, WkB, WkC = _split3_w(Wqkv[:, 1 * D + qcols], 64.0)
        WvA, WvB, WvC = _split3_w(
            Wqkv[:, 2 * D + g * HCOLS: 2 * D + (g + 1) * HCOLS], 64.0
        )
        # Wp: T1 = 32*y_head; fold 1/32 here. c=2048 keeps fp8 in normal range
        WpA, WpB, WpC = _split3_w(
            Wproj[g * HCOLS:(g + 1) * HCOLS, :] / 32.0, 2048.0
        )
        xa, xc = xsplit[b]
        in_maps.append({
            "XA": xa, "XC": xc,
            "WqA": WqA, "WqB": WqB, "WqC": WqC,
            "WkA": WkA, "WkB": WkB, "WkC": WkC,
            "WvA": WvA, "WvB": WvB, "WvC": WvC,
            "WpA": WpA, "WpB": WpB, "WpC": WpC,
        })

    results = _get_runner().run(in_maps)
    out = np.zeros((B, S, D), dtype=np.float32)
    for c in range(8):
        out[c // 4] += results[c]["out"].astype(np.float32)
    return out


# revision 42
# speedup vs baseline: 1.1187x; 1.1187x over previous
"""Trainium2 Bass kernel for nn_MultiHeadAttention (dense transformer block:
qkv proj + RoPE + causal SDPA + out proj), tensor-parallel over (batch, heads)
across 8 NeuronCores.

Sharding: 2 batches x 16 heads = 32 (b,h) pairs; core c handles batch c//4,
heads 4*(c%4)..4*(c%4)+3. Each core computes qkv for its 4 heads (from the
full x of its batch), RoPE, causal attention, and a PARTIAL output
projection (its heads' rows of Wproj); the host sums the 4 partials per
batch.

v2: all four big GEMMs (q/k/v projections and the output projection) run in
fp8e4m3 DoubleRow mode (2 contraction tiles per instruction, 0.5 cyc/row)
with a 3-term hi/lo error-compensated split:
    x @ W ~= (XA@WA + XA@WB + XC@WC) / 1024
where XA=fp8(x), XC=fp8(16*(x-XA)), WA=fp8(16*c*W), WC=fp8(c*W),
WB=fp8(16*(c*W - WC)), c=64. This keeps quantization error at bf16 level
(~0.1% per GEMM) while running the PE 1.33x faster than bf16 on those GEMMs.
Attention (scores / exp / PV) stays bf16.

Layout notes:
- x is passed pre-transposed per batch and pre-split into fp8 hi/lo (XA/XC
  [D, S]) so the contraction dim lands on SBUF partitions, streamed through
  SBUF in 512-token chunks.
- q/k head dims are permuted host-side into a 16-interleaved (even,odd)
  order so RoPE's pair swap is a quadrant-local DVE stream_shuffle.
  Attention scores are invariant to this (q and k permuted identically).
- RoPE tables carry sqrt(scale)/1024 (the fp8 descale); V stays at 1024x in
  bf16 and the 1/1024 is folded into the host-side Wproj scaling.
- Scores are computed transposed (S^T [kv, q]) so softmax's denominator
  comes from a ones-matmul (column sums) and P^T feeds the O = V^T @ P^T
  matmul directly. exp() runs without max-subtraction: |scores| < ~10 for
  this input distribution, safe in fp32.
- The attention output is renormalized into T1 = 32*y_head (ones matrix
  holds 32.0 so rinv = 1/(32*l)), then split into fp8 hi/lo (YA/YC) feeding
  the fp8 out-projection; final psum carries 32768*out, descaled by the
  ACT copy.
"""
import sys

sys.path.insert(0, "/opt/trn_rl_repo")

from collections import deque

import numpy as np
import ml_dtypes

import concourse.bass as bass
import concourse.mybir as mybir
import concourse.tile as tile

P = 128
B, S, D = 2, 2048, 2048
NH, HD = 16, 128
NH_CORE = 4  # heads per core
HCOLS = NH_CORE * HD  # 512
KT = D // P  # 16 k-tiles
KP = KT // 2  # 8 k-tile pairs (DoubleRow)
TT = S // P  # 16 token tiles
QC = 512  # q-chunk width
NQC = S // QC  # 4
ROPE_THETA = 10000.0
SCALE = HD**-0.5
NEG = -30000.0

F32 = mybir.dt.float32
BF16 = mybir.dt.bfloat16
F8 = mybir.dt.float8e4
DR = mybir.MatmulPerfMode.DoubleRow

F8NP = ml_dtypes.float8_e4m3

# fp8 scale bookkeeping:
#   qkv:   XA(1) @ WA(1024) etc -> psum = 1024 * qkv
#   rope tables carry sqrt(SCALE)/1024  -> Qt/Kt = sqrt(SCALE) * q/k
#   Vt (bf16) = 1024 * v
#   ones matrix = 32.0 -> rinv = 1/(32*l) -> T1 = o_ps*rinv = 32*y_head
#   outproj: YA(32) @ WpA(1024/32*16... ) -> psum = 32768*out
OUT_DESCALE = 1.0 / 32768.0

_SWAP16 = [(i + 16) % 32 for i in range(32)]


# ---------------------------------------------------------------------------
# host-side constant tables
# ---------------------------------------------------------------------------
def _dim_perm():
    """Permutation p -> original head-dim index, 16-interleaved even/odd."""
    perm = np.zeros(HD, dtype=np.int64)
    for p in range(HD):
        qd, sl = p // 32, p % 32
        i = 16 * qd + (sl % 16)
        perm[p] = 2 * i if sl < 16 else 2 * i + 1
    return perm


def _rope_tables():
    """ctab[p,t], stab[p,t] (sign-baked) for the permuted head-dim layout."""
    perm = _dim_perm()
    inv_freq = 1.0 / (ROPE_THETA ** (np.arange(0, HD, 2, dtype=np.float64) / HD))
    t = np.arange(S, dtype=np.float64)
    ctab = np.zeros((HD, S), dtype=np.float64)
    stab = np.zeros((HD, S), dtype=np.float64)
    for p in range(HD):
        qd, sl = p // 32, p % 32
        i = 16 * qd + (sl % 16)
        ang = t * inv_freq[i]
        ctab[p] = np.cos(ang)
        stab[p] = -np.sin(ang) if sl < 16 else np.sin(ang)
    return ctab.astype(np.float32), stab.astype(np.float32)


def _tri_mask():
    """[P, P] f32: 0 where kv(row) <= q(col) else NEG."""
    b = np.arange(P)[:, None]
    a = np.arange(P)[None, :]
    return np.where(b <= a, 0.0, NEG).astype(np.float32)


def _split3_w(W, c):
    """3-term fp8 split of weights: WA=fp8(16c*W), WB=fp8(16*(c*W-fp8(c*W))),
    WC=fp8(c*W). All numpy fp8e4m3."""
    Ws = (c * W).astype(np.float32)
    WC = Ws.astype(F8NP)
    WB = (16.0 * (Ws - WC.astype(np.float32))).astype(F8NP)
    WA = (16.0 * Ws).astype(F8NP)
    return WA, WB, WC


def _split2_x(x):
    """XA=fp8(x), XC=fp8(16*(x-XA))."""
    XA = x.astype(F8NP)
    XC = (16.0 * (x - XA.astype(np.float32))).astype(F8NP)
    return XA, XC


# ---------------------------------------------------------------------------
# device kernel
# ---------------------------------------------------------------------------
def _build_nc():
    nc = bass.Bass()

    XA = nc.declare_dram_parameter("XA", [D, S], F8, isOutput=False)
    XC = nc.declare_dram_parameter("XC", [D, S], F8, isOutput=False)
    wq = [nc.declare_dram_parameter(f"Wq{t}", [D, HCOLS], F8, isOutput=False)
          for t in "ABC"]
    wk = [nc.declare_dram_parameter(f"Wk{t}", [D, HCOLS], F8, isOutput=False)
          for t in "ABC"]
    wv = [nc.declare_dram_parameter(f"Wv{t}", [D, HCOLS], F8, isOutput=False)
          for t in "ABC"]
    wp = [nc.declare_dram_parameter(f"Wp{t}", [HCOLS, D], F8, isOutput=False)
          for t in "ABC"]
    out = nc.declare_dram_parameter("out", [S, D], BF16, isOutput=True)

    # sqrt(SCALE)/1024 on both q and k tables => scores scaled by SCALE
    ctab_np, stab_np = _rope_tables()
    rt = np.float32(np.sqrt(SCALE) / 1024.0)
    cq_d = nc.inline_tensor((ctab_np * rt).astype(ml_dtypes.bfloat16), "cq")
    sq_d = nc.inline_tensor((stab_np * rt).astype(ml_dtypes.bfloat16), "sq")
    # transposed causal mask (bf16) for the PE psum-preload trick, plus a
    # bf16 identity as its moving operand
    trimT_d = nc.inline_tensor(
        np.ascontiguousarray(_tri_mask().T).astype(ml_dtypes.bfloat16),
        "trimT",
    )
    identb_d = nc.inline_tensor(
        np.eye(P, dtype=np.float32).astype(ml_dtypes.bfloat16), "identb"
    )
    ident_d = nc.inline_tensor(np.eye(P, dtype=np.float32), "ident")
    # indicator for the rinv broadcast matmuls; carries the 1/32 T1 scale
    ind_np = np.zeros((4, QC), dtype=np.float32)
    for s in range(4):
        ind_np[s, s * P:(s + 1) * P] = 1.0 / 32.0
    ind_d = nc.inline_tensor(ind_np.astype(ml_dtypes.bfloat16), "indic")

    XA_t = XA[:].rearrange("(ko p) t -> p ko t", p=P)
    XC_t = XC[:].rearrange("(ko p) t -> p ko t", p=P)
    wq_t = [w[:].rearrange("(ko p) m -> p ko m", p=P) for w in wq]
    wk_t = [w[:].rearrange("(ko p) m -> p ko m", p=P) for w in wk]
    wv_t = [w[:].rearrange("(ko p) m -> p ko m", p=P) for w in wv]
    wp_t = [w[:].rearrange("(ho p) n -> p ho n", p=P) for w in wp]
    out_t = out[:].rearrange("(to p) n -> p to n", p=P)

    with tile.TileContext(nc) as tc:
        with (
            tc.tile_pool(name="persist", bufs=1) as pp,
            tc.tile_pool(name="work", bufs=2) as wkp,
            tc.tile_pool(name="pt", bufs=3) as ptp,
            tc.tile_pool(name="t1", bufs=1) as t1p,
            tc.tile_pool(name="qtp", bufs=2) as qtp,
            tc.tile_pool(name="xwin", bufs=2) as xw,
            tc.tile_pool(name="outp", bufs=3) as outp,
            tc.tile_pool(name="ps_g", bufs=2, space="PSUM") as psG,
            tc.tile_pool(name="ps_s", bufs=3, space="PSUM") as psS,
            tc.tile_pool(name="ps_o", bufs=2, space="PSUM") as psO,
            tc.tile_pool(name="ps_l", bufs=1, space="PSUM") as psL,
        ):
            # PE warmup: keep the tensor engine busy during the initial DMAs
            # so the p-state ramp finishes before real work starts.
            warm = pp.tile([P, P], BF16)
            nc.vector.memset(warm, 0.0)
            wps = psG.tile([P, QC], F32, tag="ps")
            for i in range(48):
                nc.tensor.matmul(wps[:, 0:P], warm, warm, start=(i == 0),
                                 stop=(i == 47))

            # persistent tiles
            cq = pp.tile([P, S], BF16)
            sq = pp.tile([P, S], BF16)
            trimT = pp.tile([P, P], BF16)
            identb = pp.tile([P, P], BF16)
            ident_sb = pp.tile([P, P], F32)
            ind_sb = pp.tile([4, QC], BF16)
            ones_col = pp.tile([P, 1], BF16)
            nc.vector.memset(ones_col, 1.0)

            Kt = pp.tile([P, NH_CORE, S], BF16)
            Vt = pp.tile([P, TT, HCOLS], BF16)
            YA = pp.tile([P, NH_CORE, S], F8)
            YC = pp.tile([P, NH_CORE, S], F8)

            wq_sb = [pp.tile([P, KT, HCOLS], F8, name=f"wq{t}")
                     for t in "ABC"]
            wk_sb = [pp.tile([P, KT, HCOLS], F8, name=f"wk{t}")
                     for t in "ABC"]
            wv_sb = [pp.tile([P, KT, HCOLS], F8, name=f"wv{t}")
                     for t in "ABC"]
            wp_sb = [pp.tile([P, NH_CORE, D], F8, name=f"wp{t}")
                     for t in "ABC"]
            # spread the initial loads across SP/Pool/ACT so the first
            # matmuls (A-term of the k-projection) can start ~3.5us in.
            # Pool stays mostly free (it drains proj psums from ~7us on);
            # ACT is free until the first exp (~23us).
            nc.gpsimd.dma_start(wk_sb[0], wk_t[0])
            nc.gpsimd.dma_start(wk_sb[2], wk_t[2])
            nc.scalar.dma_start(wk_sb[1], wk_t[1])
            nc.scalar.dma_start(cq, cq_d[:])
            nc.scalar.dma_start(sq, sq_d[:])
            nc.scalar.dma_start(wv_sb[0], wv_t[0])
            nc.scalar.dma_start(wv_sb[1], wv_t[1])
            nc.scalar.dma_start(wq_sb[1], wq_t[1])

            def load_xchunk(tcx):
                xa = xw.tile([P, KT, QC], F8, tag="xa")
                xc = xw.tile([P, KT, QC], F8, tag="xc")
                nc.sync.dma_start(xa, XA_t[:, :, tcx * QC:(tcx + 1) * QC])
                nc.sync.dma_start(xc, XC_t[:, :, tcx * QC:(tcx + 1) * QC])
                return xa, xc

            def mm3(ps, w3, x2, lhs_w, hsl, xsl):
                """24 DoubleRow matmuls accumulating the 3-term product.
                lhs_w: True if weights are the stationary operand."""
                terms = ((w3[0], x2[0]), (w3[1], x2[0]), (w3[2], x2[1]))
                n = len(terms) * KP
                i = 0
                for wt, xt in terms:
                    for kp in range(KP):
                        ks = slice(2 * kp, 2 * kp + 2)
                        if lhs_w:
                            lhsT = wt[:, ks, hsl]
                            rhs = xt[:, ks, xsl]
                        else:
                            lhsT = xt[:, ks, xsl]
                            rhs = wt[:, ks, hsl]
                        nc.tensor.matmul(
                            ps, lhsT, rhs,
                            start=(i == 0), stop=(i == n - 1),
                            perf_mode=DR,
                        )
                        i += 1

            def rope(ps, dst, csl):
                pc = wkp.tile([P, QC], BF16, tag="pc")
                nc.gpsimd.tensor_copy(pc, ps)
                xsw = wkp.tile([P, QC], BF16, tag="xsw")
                nc.vector.stream_shuffle(xsw, pc, _SWAP16)
                nc.vector.tensor_mul(pc, pc, cq[:, csl])
                nc.vector.tensor_mul(xsw, xsw, sq[:, csl])
                nc.vector.tensor_add(dst, pc, xsw)

            # softmax-denominator bank: l columns [0:4] reused every chunk
            # (the WAR overlap with the previous reciprocal read orders the
            # bank-zeroing start correctly).
            lps = psL.tile([P, QC], F32)
            chunks = [(h, qc) for qc in range(NQC) for h in range(NH_CORE)]
            o_hist, rv_hist, rvT_hist, bc_hist = {}, {}, {}, {}
            pts = {}

            def post_a(i):
                # rinv [128,4] -> transposed [4,128] block in a psG slot
                rt_ps = psG.tile([P, P], F32, tag="ps")
                nc.tensor.transpose(rt_ps[0:4], rv_hist[i], ident_sb)
                rvT = wkp.tile([P, P], BF16, tag="rvT")
                nc.scalar.activation(
                    rvT[0:4], rt_ps[0:4],
                    mybir.ActivationFunctionType.Copy,
                )
                rvT_hist[i] = rvT

            def post_b(i):
                # broadcast rinv/32 across partitions: indicator matmuls
                bc_ps = psS.tile([P, QC], F32, tag="sps")
                for s in range(4):
                    nc.tensor.matmul(
                        bc_ps[:, s * P:(s + 1) * P],
                        ind_sb[0:4, s * P:(s + 1) * P],
                        rvT_hist[i][0:4],
                        start=(s == 0),
                        stop=(s == 3),
                        skip_group_check=True,
                    )
                bc = wkp.tile([P, QC], BF16, tag="bc", bufs=1)
                nc.gpsimd.tensor_copy(bc, bc_ps)
                bc_hist[i] = bc

            def post_c(i):
                h, qc = chunks[i]
                qsl = slice(qc * QC, (qc + 1) * QC)
                # T1 = 32 * y_head (f32), then fp8 hi/lo split
                t1 = t1p.tile([P, QC], F32, tag="t1")
                nc.vector.tensor_mul(t1, o_hist[i], bc_hist[i])
                nc.gpsimd.tensor_copy(YA[:, h, qsl], t1)
                nc.vector.tensor_sub(t1, t1, YA[:, h, qsl])
                nc.gpsimd.tensor_scalar_mul(YC[:, h, qsl], t1, 16.0)

            # Cross-chunk software pipeline: scores/exp run ~3 jb ahead of
            # PV/l so the mask+exp latency hides under later scores matmuls.
            # `pending` holds (chunk, jb) pairs whose PV/l is not yet
            # emitted; `after_pop` holds the deferred rinv post-chain steps,
            # one drained per pop so they spread between PV matmuls.
            pending = deque()
            after_pop = deque()

            def pv_l(i, jb):
                h, qc = chunks[i]
                njb = 4 * qc + 4
                d = jb - 4 * qc
                off = 128 * d if d > 0 else 0
                pt = pts.pop((i, jb))
                nc.tensor.matmul(
                    o_hist[i][:, off:],
                    Vt[:, jb, h * HD:(h + 1) * HD],
                    pt[:, off:],
                    start=(jb == 0),
                    stop=(jb == njb - 1),
                )
                for s in range(max(d, 0), 4):
                    nc.tensor.matmul(
                        lps[:, s:s + 1],
                        pt[:, s * P:(s + 1) * P],
                        ones_col,
                        start=(jb == 0 and s == 0),
                        stop=(jb == 4 * qc + s),
                        skip_group_check=True,
                    )

            def pop_one():
                i, jb = pending.popleft()
                pv_l(i, jb)
                h, qc = chunks[i]
                if jb == 4 * qc + 3:  # chunk complete: reciprocal + posts
                    rv = wkp.tile([P, 4], F32, tag="rv")
                    nc.vector.reciprocal(rv, lps[:, 0:4])
                    rv_hist[i] = rv
                    after_pop.append(lambda i=i: post_a(i))
                    after_pop.append(lambda i=i: post_b(i))
                    after_pop.append(lambda i=i: post_c(i))
                elif after_pop:
                    after_pop.popleft()()

            def flush():
                while pending:
                    pop_one()
                while after_pop:
                    after_pop.popleft()()

            def attention_chunk(i, qtile):
                h, qc = chunks[i]
                o_hist[i] = psO.tile([P, QC], F32, tag="ops", name="o_ps")
                njb = 4 * qc + 4
                for jb in range(njb):
                    d = jb - 4 * qc  # diag offset if >= 0
                    off = 128 * d if d > 0 else 0
                    s_ps = psS.tile([P, QC], F32, tag="sps")
                    if d >= 0:
                        # causal mask preloaded into psum by the PE itself
                        # (start zeroes the bank, then writes the -30000s)
                        nc.tensor.matmul(
                            s_ps[:, off:off + P], trimT, identb,
                            start=True, stop=False, skip_group_check=True,
                        )
                        nc.tensor.matmul(
                            s_ps[:, off:],
                            Kt[:, h, jb * P:(jb + 1) * P],
                            qtile[:, h, off:],
                            start=False, stop=True, skip_group_check=True,
                        )
                    else:
                        nc.tensor.matmul(
                            s_ps[:, off:],
                            Kt[:, h, jb * P:(jb + 1) * P],
                            qtile[:, h, off:],
                            start=True, stop=True,
                        )
                    pt = ptp.tile([P, QC], BF16, tag="pt")
                    nc.scalar.activation(
                        pt[:, off:],
                        s_ps[:, off:],
                        mybir.ActivationFunctionType.Exp,
                    )
                    pts[(i, jb)] = pt
                    pending.append((i, jb))
                    if len(pending) > 2:
                        pop_one()

            def outproj_tt(tt):
                tsl = slice(tt * P, (tt + 1) * P)
                for ncx in range(D // QC):
                    nsl = slice(ncx * QC, (ncx + 1) * QC)
                    ps = psG.tile([P, QC], F32, tag="ps", name="op_ps")
                    terms = ((YA, wp_sb[0]), (YA, wp_sb[1]), (YC, wp_sb[2]))
                    i = 0
                    for yt, wt in terms:
                        for hp in range(NH_CORE // 2):
                            hs = slice(2 * hp, 2 * hp + 2)
                            nc.tensor.matmul(
                                ps,
                                yt[:, hs, tsl],
                                wt[:, hs, nsl],
                                start=(i == 0),
                                stop=(i == 5),
                                perf_mode=DR,
                            )
                            i += 1
                    ob = outp.tile([P, QC], BF16, tag="ob")
                    nc.gpsimd.tensor_scalar_mul(ob, ps, float(OUT_DESCALE))
                    nc.sync.dma_start(out_t[:, tt, nsl], ob)

            def kproj(h, x2, csl):
                ps = psG.tile([P, QC], F32, tag="ps")
                mm3(ps, wk_sb, x2, True,
                    slice(h * HD, (h + 1) * HD), slice(None))
                rope(ps, Kt[:, h, csl], csl)

            def vproj(tt, x2):
                sub = tt % 4
                ps = psG.tile([P, HCOLS], F32, tag="ps")
                mm3(ps, wv_sb, x2, False,
                    slice(None), slice(sub * P, (sub + 1) * P))
                nc.gpsimd.tensor_copy(Vt[:, tt], ps)

            def qproj(h, x2, qtile, csl):
                ps = psG.tile([P, QC], F32, tag="ps")
                mm3(ps, wq_sb, x2, True,
                    slice(h * HD, (h + 1) * HD), slice(None))
                rope(ps, qtile[:, h], csl)

            # ---- systolic merge: attention(tcx) x projections(tcx+1) x ----
            # ---- out-projection(tcx-1), one head per step -----------------
            # chunk (h, qc) needs K/V tiles only up to tcx=qc and Q(h, qc),
            # so PE-heavy projections for tcx+1 pad the ACT(exp)-bound
            # attention stretches of tcx; the out-projection trails one tcx.
            ci = 0
            xa_c, xc_c = load_xchunk(0)
            # remaining prologue loads, behind x chunk 0 on SP
            nc.sync.dma_start(ident_sb, ident_d[:])
            nc.sync.dma_start(trimT, trimT_d[:])
            nc.sync.dma_start(identb, identb_d[:])
            nc.sync.dma_start(ind_sb[0:4], ind_d[:])
            nc.sync.dma_start(wv_sb[2], wv_t[2])
            nc.sync.dma_start(wq_sb[2], wq_t[2])
            nc.sync.dma_start(wq_sb[0], wq_t[0])
            for i in range(3):
                nc.sync.dma_start(wp_sb[i], wp_t[i])

            # pipeline fill: projections for tcx=0
            csl0 = slice(0, QC)
            qtile_c = qtp.tile([P, NH_CORE, QC], BF16, tag="qt")
            for h in range(NH_CORE):
                kproj(h, (xa_c, xc_c), csl0)
            for tt in range(4):
                vproj(tt, (xa_c, xc_c))
            for h in range(NH_CORE):
                qproj(h, (xa_c, xc_c), qtile_c, csl0)

            for tcx in range(NQC):
                if tcx + 1 < NQC:
                    xa_n, xc_n = load_xchunk(tcx + 1)
                    csl_n = slice((tcx + 1) * QC, (tcx + 2) * QC)
                    qtile_n = qtp.tile([P, NH_CORE, QC], BF16, tag="qt",
                                       name="qtile")
                for h in range(NH_CORE):
                    attention_chunk(ci, qtile_c)
                    ci += 1
                    if tcx + 1 < NQC:
                        kproj(h, (xa_n, xc_n), csl_n)
                        vproj(4 * (tcx + 1) + h, (xa_n, xc_n))
                        qproj(h, (xa_n, xc_n), qtile_n, csl_n)
                    if tcx > 0:
                        outproj_tt(4 * (tcx - 1) + h)
                if tcx + 1 < NQC:
                    xa_c, xc_c = xa_n, xc_n
                    qtile_c = qtile_n
            flush()
            for tt in range(4 * (NQC - 1), TT):  # last qc's out-projection
                outproj_tt(tt)
    return nc


# ---------------------------------------------------------------------------
# legalization: this walrus build supports only ONE sync wait per instruction
# ---------------------------------------------------------------------------
_ENGINE_SEM_PREFIX = {
    "PE": "PE_",
    "DVE": "DVE_",
    "ACT": "ACT_",
    "Pool": "POOL_",
    "SP": "SP_",
}
_wf_counter = [0]


def _legalize(nc, max_waits=1):
    for f in nc.m.functions:
        for bb in f.blocks:
            new_insts = []
            for inst in bb.instructions:
                si = getattr(inst, "sync_info", None)
                eng = getattr(inst, "engine", None)
                if si is None or not si.on_wait or eng is None:
                    new_insts.append(inst)
                    continue
                waits = list(si.on_wait)
                pref = _ENGINE_SEM_PREFIX.get(eng.name)
                if pref is not None:
                    waits = [
                        w
                        for w in waits
                        if not (
                            w.sync_type == "semaphore"
                            and w.ant_name.startswith(pref)
                        )
                    ]
                if len(waits) > max_waits:
                    for w in waits[:-max_waits]:
                        _wf_counter[0] += 1
                        nop = mybir.InstNoOp(
                            name=f"I-waitfix-{_wf_counter[0]}", ins=[], outs=[]
                        )
                        nop.engine = eng
                        nop.sync_info = mybir.SyncInfo(on_wait=[w], on_update=[])
                        new_insts.append(nop)
                    waits = waits[-max_waits:]
                if len(waits) != len(si.on_wait):
                    inst.sync_info = mybir.SyncInfo(
                        on_wait=waits, on_update=list(si.on_update)
                    )
                new_insts.append(inst)
            bb.instructions[:] = new_insts


# ---------------------------------------------------------------------------
# SPMD runner (mirrors concourse.bass2jax.run_bass_via_pjrt, kept resident)
# ---------------------------------------------------------------------------
class _Runner:
    def __init__(self, nc, n_cores=8):
        import jax
        from jax.sharding import Mesh, PartitionSpec
        from jax.experimental.shard_map import shard_map
        from concourse import bass2jax
        from concourse.bass2jax import _bass_exec_p, install_neuronx_cc_hook

        install_neuronx_cc_hook()
        self.jax = jax
        self.nc = nc
        self.n_cores = n_cores
        partition_name = (
            nc.partition_id_tensor.name if nc.partition_id_tensor else None
        )
        in_names, out_names, out_avals, zero_outs = [], [], [], []
        for alloc in nc.m.functions[0].allocations:
            if not isinstance(alloc, mybir.MemoryLocationSet):
                continue
            name = alloc.memorylocations[0].name
            if alloc.kind == "ExternalInput":
                if name != partition_name:
                    in_names.append(name)
            elif alloc.kind == "ExternalOutput":
                shape = tuple(alloc.tensor_shape)
                dtype = mybir.dt.np(alloc.dtype)
                out_names.append(name)
                out_avals.append(jax.core.ShapedArray(shape, dtype))
                zero_outs.append(np.zeros(shape, dtype))
        self.in_names, self.out_names = in_names, out_names
        self.out_avals, self.zero_outs = out_avals, zero_outs
        n_params, n_outs = len(in_names), len(out_names)
        all_in_names = in_names + out_names
        if partition_name is not None:
            all_in_names.append(partition_name)
        donate = tuple(range(n_params, n_params + n_outs))

        def _body(*args):
            operands = list(args)
            if partition_name is not None:
                operands.append(bass2jax.partition_id_tensor())
            return tuple(
                _bass_exec_p.bind(
                    *operands,
                    out_avals=tuple(out_avals),
                    in_names=tuple(all_in_names),
                    out_names=tuple(out_names),
                    lowering_input_output_aliases=(),
                    sim_require_finite=True,
                    sim_require_nnan=True,
                    nc=nc,
                )
            )

        devices = jax.devices()[:n_cores]
        mesh = Mesh(np.asarray(devices), ("core",))
        in_specs = (PartitionSpec("core"),) * (n_params + n_outs)
        out_specs = (PartitionSpec("core"),) * n_outs
        self.fn = jax.jit(
            shard_map(
                _body,
                mesh=mesh,
                in_specs=in_specs,
                out_specs=out_specs,
                check_rep=False,
            ),
            donate_argnums=donate,
            keep_unused=True,
        )

    def run(self, in_maps):
        n = self.n_cores
        concat_in = [
            np.concatenate(
                [np.asarray(in_maps[c][name]) for c in range(n)], axis=0
            )
            for name in self.in_names
        ]
        zeros = [
            np.zeros((n * z.shape[0], *z.shape[1:]), z.dtype)
            for z in self.zero_outs
        ]
        out_arrs = self.fn(*concat_in, *zeros)
        return [
            {
                name: np.asarray(out_arrs[i]).reshape(
                    n, *self.out_avals[i].shape
                )[c]
                for i, name in enumerate(self.out_names)
            }
            for c in range(n)
        ]


_RUNNER = None


def _get_runner():
    global _RUNNER
    if _RUNNER is None:
        nc = _build_nc()
        _legalize(nc)
        _RUNNER = _Runner(nc, 8)
    return _RUNNER


# ---------------------------------------------------------------------------
# public entry point
# ---------------------------------------------------------------------------
def kernel(x, Wqkv, Wproj):
    x = np.asarray(x, dtype=np.float32)
    Wqkv = np.asarray(Wqkv, dtype=np.float32)
    Wproj = np.asarray(Wproj, dtype=np.float32)
    perm = _dim_perm()

    xsplit = [_split2_x(np.ascontiguousarray(x[b].T)) for b in range(B)]
    in_maps = []
    for c in range(8):
        b, g = c // 4, c % 4
        heads = range(NH_CORE * g, NH_CORE * (g + 1))
        qcols = np.concatenate([h * HD + perm for h in heads])
        WqA, WqB, WqC = _split3_w(Wqkv[:, 0 * D + qcols], 64.0)
        WkA, WkB, WkC = _split3_w(Wqkv[:, 1 * D + qcols], 64.0)
        WvA, WvB, WvC = _split3_w(
            Wqkv[:, 2 * D + g * HCOLS: 2 * D + (g + 1) * HCOLS], 64.0
        )
        # Wp: T1 = 32*y_head; fold 1/32 here. c=2048 keeps fp8 in normal range
        WpA, WpB, WpC = _split3_w(
            Wproj[g * HCOLS:(g + 1) * HCOLS, :] / 32.0, 2048.0
        )
        xa, xc = xsplit[b]
        in_maps.append({
            "XA": xa, "XC": xc,
            "WqA": WqA, "WqB": WqB, "WqC": WqC,
            "WkA": WkA, "WkB": WkB, "WkC": WkC,
            "WvA": WvA, "WvB": WvB, "WvC": WvC,
            "WpA": WpA, "WpB": WpB, "WpC": WpC,
        })

    results = _get_runner().run(in_maps)
    out = np.zeros((B, S, D), dtype=np.float32)
    for c in range(8):
        out[c // 4] += results[c]["out"].astype(np.float32)
    return out
